# revision 4
# baseline (speedup 1.0000x reference)
"""Cumsum along axis=2 of a (64, 256, 1024, 4) f32 tensor on 8 TRN2 NeuronCores.

Strategy: trivially data-parallel over the batch axis (8 batches per core).
Per core the shard is viewed as (2048 rows, 4096 cols) where each row is one
(b, c) slice laid out as [t0s0 t0s1 t0s2 t0s3 t1s0 ...].  The inclusive prefix
sum over t (stride-4 groups) is computed with the DVE's native
TensorTensorScan instruction: 4 scans per tile, one per stream s, each over a
stride-4 access pattern of length 1024.  All HBM traffic is fully contiguous
2MB tiles, so the kernel is DMA-bound at the memory roofline.
"""

import numpy as np

import concourse.bacc as bacc
import concourse.mybir as mybir
from concourse import tile
from concourse.bass_utils import run_bass_kernel_spmd

N_CORES = 8
B, C, T, S = 64, 256, 1024, 4
B_PER_CORE = B // N_CORES          # 8
ROWS = B_PER_CORE * C              # 2048 independent (b, c) rows per core
FREE = T * S                       # 4096 f32 per row
P = 128                            # SBUF partitions
N_TILES = ROWS // P                # 16 tiles of (128, 4096) per core

_nc_cache = None


def _build():
    nc = bacc.Bacc("TRN2", target_bir_lowering=False, debug=False)
    x = nc.dram_tensor("x", [ROWS, FREE], mybir.dt.float32, kind="ExternalInput").ap()
    y = nc.dram_tensor("y", [ROWS, FREE], mybir.dt.float32, kind="ExternalOutput").ap()

    add = mybir.AluOpType.add
    with tile.TileContext(nc) as tc:
        with (
            tc.tile_pool(name="const", bufs=1) as cpool,
            tc.tile_pool(name="in", bufs=3) as in_pool,
            tc.tile_pool(name="out", bufs=3) as out_pool,
        ):
            # data0 operand for the scan recurrence: state = (0 + state) + x_t
            zeros = cpool.tile([P, T], mybir.dt.float32)
            nc.vector.memset(zeros[:], 0.0)

            for i in range(N_TILES):
                tin = in_pool.tile([P, FREE], mybir.dt.float32, tag="tin")
                nc.sync.dma_start(tin[:], x[i * P : (i + 1) * P, :])
                tout = out_pool.tile([P, FREE], mybir.dt.float32, tag="tout")
                for s in range(S):
                    nc.vector.tensor_tensor_scan(
                        tout[:, s::S],
                        zeros[:],
                        tin[:, s::S],
                        0.0,
                        add,
                        add,
                    )
                nc.sync.dma_start(y[i * P : (i + 1) * P, :], tout[:])
    nc.compile()
    return nc


def _get_nc():
    global _nc_cache
    if _nc_cache is None:
        _nc_cache = _build()
    return _nc_cache


def kernel(x: np.ndarray) -> np.ndarray:
    x = np.ascontiguousarray(np.asarray(x, dtype=np.float32))
    assert x.shape == (B, C, T, S), x.shape
    shards = x.reshape(N_CORES, ROWS, FREE)
    in_maps = [{"x": shards[k]} for k in range(N_CORES)]
    res = run_bass_kernel_spmd(_get_nc(), in_maps, core_ids=list(range(N_CORES)))
    out = np.stack([res.results[k]["y"] for k in range(N_CORES)], axis=0)
    return out.reshape(B, C, T, S)


# revision 12
# speedup vs baseline: 25.0346x; 25.0346x over previous
"""Cumsum along axis=2 of a (64, 256, 1024, 4) f32 tensor on 8 TRN2 NeuronCores.

Strategy: trivially data-parallel over the batch axis (8 batches per core).
Per core the shard is viewed as (2048 rows, 4096 cols) where each row is one
(b, c) slice laid out as [t0s0 t0s1 t0s2 t0s3 t1s0 ...].  The inclusive prefix
sum over t (stride-4 groups) is computed with the DVE's native
TensorTensorScan instruction: 4 scans per tile, one per stream s, each over a
stride-4 access pattern of length 1024.  All HBM traffic is fully contiguous
2MB tiles, so the kernel is DMA-bound at the memory roofline.
"""

import numpy as np

import concourse.bacc as bacc
import concourse.mybir as mybir
from concourse import tile
from concourse.bass_utils import run_bass_kernel_spmd

N_CORES = 8
B, C, T, S = 64, 256, 1024, 4
B_PER_CORE = B // N_CORES          # 8
ROWS = B_PER_CORE * C              # 2048 independent (b, c) rows per core
FREE = T * S                       # 4096 f32 per row
P = 128                            # SBUF partitions
N_TILES = ROWS // P                # 16 tiles of (128, 4096) per core

_nc_cache = None


def _build(
    repeat: int = 1,
    scan: bool = True,
    bufs: int = 3,
    blocks_per_tile: int = 1,
    store_engine: str = "sync",
    inplace: bool = False,
):
    """blocks_per_tile: how many 128-row blocks one SBUF tile (and one DMA
    transfer) covers; free dim = blocks_per_tile * 4096."""
    nc = bacc.Bacc("TRN2", target_bir_lowering=False, debug=False)
    x = nc.dram_tensor("x", [ROWS, FREE], mybir.dt.float32, kind="ExternalInput").ap()
    y = nc.dram_tensor("y", [ROWS, FREE], mybir.dt.float32, kind="ExternalOutput").ap()

    add = mybir.AluOpType.add
    nb = blocks_per_tile
    n_tiles = N_TILES // nb
    tile_free = nb * FREE
    with tile.TileContext(nc) as tc:
        with (
            tc.tile_pool(name="const", bufs=1) as cpool,
            tc.tile_pool(name="in", bufs=bufs) as in_pool,
            tc.tile_pool(name="out", bufs=bufs) as out_pool,
        ):
            # data0 operand for the scan recurrence: state = (0 + state) + x_t
            zeros = cpool.tile([P, T], mybir.dt.float32)
            nc.vector.memset(zeros[:], 0.0)

            store = getattr(nc, store_engine)
            for _ in range(repeat):
                for i in range(n_tiles):
                    # x rows [i*nb*P, (i+1)*nb*P) viewed as [P, (nb, FREE)]:
                    # partition p holds rows i*nb*P + j*P + p for j in range(nb).
                    src = x[i * nb * P : (i + 1) * nb * P, :].rearrange(
                        "(n p) f -> p n f", p=P
                    )
                    dst = y[i * nb * P : (i + 1) * nb * P, :].rearrange(
                        "(n p) f -> p n f", p=P
                    )
                    tin = in_pool.tile([P, tile_free], mybir.dt.float32, tag="tin")
                    nc.sync.dma_start(tin[:].rearrange("p (n f) -> p n f", n=nb), src)
                    if scan == "passthrough":
                        store.dma_start(
                            dst, tin[:].rearrange("p (n f) -> p n f", n=nb)
                        )
                        continue
                    if inplace:
                        tout = tin
                    else:
                        tout = out_pool.tile(
                            [P, tile_free], mybir.dt.float32, tag="tout"
                        )
                    if scan:
                        for j in range(nb):
                            for s in range(S):
                                lo, hi = j * FREE + s, (j + 1) * FREE
                                nc.vector.tensor_tensor_scan(
                                    tout[:, lo:hi:S],
                                    zeros[:],
                                    tin[:, lo:hi:S],
                                    0.0,
                                    add,
                                    add,
                                )
                    else:
                        nc.vector.tensor_copy(tout[:], tin[:])
                    store.dma_start(
                        dst, tout[:].rearrange("p (n f) -> p n f", n=nb)
                    )
    nc.compile()
    return nc


def _get_nc():
    global _nc_cache
    if _nc_cache is None:
        _nc_cache = _build()
    return _nc_cache


def kernel(x: np.ndarray) -> np.ndarray:
    x = np.ascontiguousarray(np.asarray(x, dtype=np.float32))
    assert x.shape == (B, C, T, S), x.shape
    shards = x.reshape(N_CORES, ROWS, FREE)
    in_maps = [{"x": shards[k]} for k in range(N_CORES)]
    res = run_bass_kernel_spmd(_get_nc(), in_maps, core_ids=list(range(N_CORES)))
    out = np.stack([res.results[k]["y"] for k in range(N_CORES)], axis=0)
    return out.reshape(B, C, T, S)
